# revision 20
# baseline (speedup 1.0000x reference)
"""Trainium2 Bass kernel for nn_CaptioningRNN (attention-LSTM).

Strategy
--------
Data-parallel over batch: 1024 rows -> 128 per core (one partition tile).
All recurrent weights live RESIDENT in SBUF in fp16 (Wh 8MiB + Wattn 8MiB +
Af 4MiB), which removes the per-timestep 32MiB weight re-stream that makes
this problem memory-bound.

Phase A: xa[t] = x_t @ Wx + b for all t in one batched pass (PE), stored to
         internal DRAM as fp16.
Phase B: 64 sequential steps, per step:
  - hT via 8 DMA-xbar transposes (no PSUM/PE cost)
  - scores_l = sum_h h*Af_l / sqrt(H) via fused tensor_tensor_reduce (DVE)
  - softmax (no max-subtraction needed: |scores| <= ~32, exp is safe)
  - attn = sum_l w_l * Af_l via per-partition tensor_scalar muls (DVE)
  - a = xa + h@Wh + attn@Wattn in PSUM: 128 fp16 matmuls + identity-inject
  - gates: sigmoid(z) = 0.5*tanh(z/2)+0.5 so ACT only needs the
    exp_and_others table set (tanh + exp), avoiding table switches
  - c = f*c + i*g ; h = o*tanh(c)  (fp32 state)

Numerics: fp16 weights/activations with fp32 PSUM/state gives rel-l2 ~4e-4
vs the fp32 reference (validated offline).
"""

import sys

for _p in ("/opt/trn_rl_repo",):
    if _p not in sys.path:
        sys.path.insert(0, _p)

import numpy as np
from contextlib import ExitStack

import concourse.bacc as bacc
import concourse.mybir as mybir
import concourse.tile as tile
from concourse.bass_utils import run_bass_kernel_spmd

NCORES = 8
N, T, D, H = 1024, 64, 512, 1024
NB = N // NCORES        # 128 batch rows per core
FH = 4 * H              # 4096
KH = H // 128           # 8 contraction chunks over H
KD = D // 128           # 4 contraction chunks over D
NL = 16                 # attention cells
SCALE = 1.0 / float(np.sqrt(H))
f16, f32 = mybir.dt.float16, mybir.dt.float32
AX = mybir.AxisListType
OP = mybir.AluOpType
ACTF = mybir.ActivationFunctionType


def _emit(ctx, tc, nc, d, T_steps):
    """Emit the full program. d: dict of dram tensor handles."""
    # ---- persistent state (small) ----
    state = ctx.enter_context(tc.tile_pool(name="state", bufs=1))
    h_sb = state.tile([NB, H], f16, tag="h")
    c_sb = state.tile([NB, H], f32, tag="c")
    id16_sb = state.tile([128, 128], f16, tag="id16")
    nc.sync.dma_start(id16_sb[:], d["idf16"][:, :])

    # ---- phase A: xa[t] = x_t @ Wx + b, stored fp16 to DRAM ----
    with tc.tile_pool(name="phA", bufs=1) as pa, \
         tc.tile_pool(name="phA_ps", bufs=2, space="PSUM") as pap, \
         tc.tile_pool(name="phA_db", bufs=2) as padb:
        wx_sb = []
        for k in range(KD):
            tw = pa.tile([128, FH], f16, tag=f"wx{k}")
            nc.sync.dma_start(tw[:], d["wx"][k * 128:(k + 1) * 128, :])
            wx_sb.append(tw)
        b_sb = pa.tile([1, FH], f16, tag="b")
        nc.sync.dma_start(b_sb[:], d["bvec"][:, :])
        ones_sb = pa.tile([1, 128], f16, tag="ones")
        nc.sync.dma_start(ones_sb[:], d["ones1"][:, :])

        for t in range(T_steps):
            xt = padb.tile([128, KD, 128], f16, tag="xt")
            for k in range(KD):
                nc.sync.dma_start(xt[:, k, :], d["xT"][t, k * 128:(k + 1) * 128, :])
            for half in range(2):
                ps = pap.tile([128, FH // 2], f32, tag="paps")
                c0 = half * (FH // 2)
                for j in range(4):
                    js = slice(j * 512, (j + 1) * 512)
                    for k in range(KD):
                        nc.tensor.matmul(ps[:, js], xt[:, k, :],
                                         wx_sb[k][:, c0 + j * 512: c0 + (j + 1) * 512],
                                         start=(k == 0), stop=False)
                    nc.tensor.matmul(ps[:, js], ones_sb[:],
                                     b_sb[:, c0 + j * 512: c0 + (j + 1) * 512],
                                     start=False, stop=True)
                xae = padb.tile([128, FH // 2], f16, tag="xae")
                nc.vector.tensor_copy(xae[:], ps[:])
                nc.sync.dma_start(d["xa_d"][t, :, c0:c0 + FH // 2], xae[:])

    # ---- resident weights (after phase A pool is released) ----
    res = ctx.enter_context(tc.tile_pool(name="resident", bufs=1))
    wh_sb, wa_sb = [], []
    for k in range(KH):
        tw = res.tile([128, FH], f16, tag=f"wh{k}")
        nc.sync.dma_start(tw[:], d["wh"][k * 128:(k + 1) * 128, :])
        wh_sb.append(tw)
    for k in range(KH):
        tw = res.tile([128, FH], f16, tag=f"wa{k}")
        nc.sync.dma_start(tw[:], d["wa"][k * 128:(k + 1) * 128, :])
        wa_sb.append(tw)
    # AfT: per h-chunk [128h, (n,l)] fp16 — PE Gram-scores operand
    afT_sb = []
    for k in range(KH):
        tw = res.tile([128, NB * NL], f16, tag=f"afT{k}")
        nc.sync.dma_start(tw[:], d["afT"][k, :, :])
        afT_sb.append(tw)
    gmask_sb = res.tile([128, 32 * NL], f16, tag="gmask")
    nc.sync.dma_start(gmask_sb[:], d["gmask"][:, :])

    # ---- phase B pools ----
    rec = ctx.enter_context(tc.tile_pool(name="rec", bufs=1))
    rec2 = ctx.enter_context(tc.tile_pool(name="rec2", bufs=1))
    afs = ctx.enter_context(tc.tile_pool(name="afs", bufs=2))
    dgp = ctx.enter_context(tc.tile_pool(name="dgp", bufs=NL))
    htp = ctx.enter_context(tc.tile_pool(name="htp", bufs=2))
    rps = ctx.enter_context(tc.tile_pool(name="rps", bufs=1, space="PSUM"))

    sc = rec.tile([NB, NL], f32, tag="sc")
    tiof = rec.tile([NB, 2 * H], f16, tag="tiof")
    attnT = rec.tile([128, KH, 128], f16, tag="attnT")

    # ---- h0 = c0 = mean_l Af  (from the streamed n-partition Af tiles) ----
    h0f = rec2.tile([NB, H], f32, tag="gt4")
    for k in range(KH):
        af_k = afs.tile([NB, NL, 128], f16, tag="afk")
        nc.sync.dma_start(af_k[:], d["af_str"][k, :, :, :])
        ks = slice(k * 128, (k + 1) * 128)
        nc.vector.tensor_reduce(
            h0f[:, ks], af_k[:, :, :].rearrange("p l h -> p h l"),
            axis=AX.X, op=OP.add)
    nc.vector.tensor_scalar_mul(h_sb[:], h0f[:], 1.0 / NL)
    nc.scalar.mul(c_sb[:], h0f[:], 1.0 / NL)
    hT = htp.tile([128, KH, 128], f16, tag="hT")
    nc.sync.dma_start_transpose(hT[:, 0:KH // 2, :], h_sb[:, 0:H // 2])
    nc.sync.dma_start_transpose(hT[:, KH // 2:KH, :], h_sb[:, H // 2:H])

    H3 = 3 * H
    for t in range(T_steps):
        # prefetch xa_t and the step's Af tiles on the SWDGE (gpsimd) queue
        xa_sb = rec2.tile([NB, FH], f16, tag="xa")
        nc.gpsimd.dma_start(xa_sb[:], d["xa_d"][t, :, :])
        af_t = []
        for k in range(KH):
            af_k = afs.tile([NB, NL, 128], f16, tag="afk")
            nc.gpsimd.dma_start(af_k[:], d["af_str"][k, :, :, :])
            af_t.append(af_k)

        # --- scores via col-tiled PE Gram: out[n, (n',l)] = sum_h h Af ---
        gram_ps = rps.tile([128, 32 * NL], f32, tag="psB")
        nc.tensor.matmul(gram_ps[0:64, 0:64], id16_sb[:, 0:64], hT[:, 0, 0:64],
                         start=True, stop=True, skip_group_check=True)
        for k in range(KH):
            for gq in range(4):
                gp = slice(gq * 32, (gq + 1) * 32)
                nc.tensor.matmul(gram_ps[gp, :], hT[:, k, gp],
                                 afT_sb[k][:, gq * 512:(gq + 1) * 512],
                                 start=(k == 0), stop=(k == KH - 1),
                                 tile_position=(0, gq * 32),
                                 skip_group_check=True)

        # --- a-accumulation, low 6 banks (i,f,o), h@Wh while DVE extracts ---
        a_lo = rps.tile([NB, H3], f32, tag="a_lo")
        for k in range(KH):
            for j in range(H3 // 512):
                js = slice(j * 512, (j + 1) * 512)
                nc.tensor.matmul(a_lo[:, js], hT[:, k, :], wh_sb[k][:, js],
                                 start=(k == 0), stop=False)

        # --- extract scores diagonal + softmax (DVE/ACT) ---
        gext = rec2.tile([128, 32 * NL], f16, tag="gext")
        nc.vector.tensor_mul(gext[:], gram_ps[:, :], gmask_sb[:])
        nc.vector.tensor_reduce(
            sc[:], gext[:, :].rearrange("p (n l) -> p l n", l=NL),
            axis=AX.X, op=OP.add)
        nc.scalar.activation(sc[:], sc[:], ACTF.Exp, scale=SCALE)
        zs = rec2.tile([NB, 1], f32, tag="zs")
        nc.vector.reduce_sum(zs[:], sc[:], axis=AX.X)
        nc.vector.reciprocal(zs[:], zs[:])
        wgt = sc
        nc.vector.tensor_scalar_mul(wgt[:], sc[:], zs[:])
        # --- attn^T directly on PE: attnT[k][h,n] = sum_l Af_k[:,l,:].T @ diag_l
        attnT_ps = rps.tile([128, KH, 128], f32, tag="psB")
        diags = []
        for l in range(NL):
            dg = dgp.tile([128, 128], f16, tag="diag")
            nc.vector.tensor_scalar_mul(dg[:], id16_sb[:], wgt[:, l:l + 1])
            diags.append(dg)
        for k in range(KH):
            for l in range(NL):
                nc.tensor.matmul(attnT_ps[:, k, :], af_t[k][:, l, :], diags[l][:],
                                 start=(l == 0), stop=(l == NL - 1))
            if k % 2 == 0:
                nc.scalar.copy(attnT[:, k, :], attnT_ps[:, k, :])
            else:
                nc.vector.tensor_copy(attnT[:, k, :], attnT_ps[:, k, :])

        # --- attn@Wattn into low banks, then the high (g-gate) 2 banks ---
        for j in range(H3 // 512):
            js = slice(j * 512, (j + 1) * 512)
            nc.tensor.matmul(a_lo[:, js], id16_sb[:], xa_sb[:, js],
                             start=False, stop=False)
        for k in range(KH):
            for j in range(H3 // 512):
                js = slice(j * 512, (j + 1) * 512)
                nc.tensor.matmul(a_lo[:, js], attnT[:, k, :], wa_sb[k][:, js],
                                 start=False, stop=(k == KH - 1))
        a_hi = rps.tile([NB, H], f32, tag="psB")
        for k in range(KH):
            nc.tensor.matmul(a_hi[:, 0:512], hT[:, k, :], wh_sb[k][:, H3:H3 + 512],
                             start=(k == 0), stop=False)
            nc.tensor.matmul(a_hi[:, 512:1024], hT[:, k, :], wh_sb[k][:, H3 + 512:FH],
                             start=(k == 0), stop=False)
        nc.tensor.matmul(a_hi[:, 0:512], id16_sb[:], xa_sb[:, H3:H3 + 512],
                         start=False, stop=False)
        nc.tensor.matmul(a_hi[:, 512:1024], id16_sb[:], xa_sb[:, H3 + 512:FH],
                         start=False, stop=False)
        for k in range(KH):
            nc.tensor.matmul(a_hi[:, 0:512], attnT[:, k, :], wa_sb[k][:, H3:H3 + 512],
                             start=False, stop=(k == KH - 1))
            nc.tensor.matmul(a_hi[:, 512:1024], attnT[:, k, :], wa_sb[k][:, H3 + 512:FH],
                             start=False, stop=(k == KH - 1))

        # gates, evacuated per gate (i,f,o: sigmoid via 0.5*tanh(z/2)+0.5)
        def dummy_mm(dep_ap, region):
            # near-free matmul keeping the PE HAM-active through gate windows;
            # writes a fully-consumed slice of a_lo as its own psum group
            nc.tensor.matmul(a_lo[0:64, region: region + 64],
                             id16_sb[:, 0:64], dep_ap,
                             start=True, stop=True, skip_group_check=True)

        for gi in (1, 0):                         # f then i (from a_lo)
            gs = slice(gi * H, (gi + 1) * H)
            nc.scalar.activation(tiof[:, gs], a_lo[:, gs], ACTF.Tanh, scale=0.5)
            nc.vector.tensor_scalar(tiof[:, gs], tiof[:, gs], 0.5, 0.5,
                                    OP.mult, OP.add)
            dummy_mm(tiof[:, gi * H: gi * H + 64], gi * H)
        ig = rec2.tile([NB, H], f32, tag="ig4")
        nc.vector.tensor_mul(ig[:], tiof[:, H:2 * H], c_sb[:])      # f*c
        g = rec2.tile([NB, H], f32, tag="gt4")
        nc.scalar.activation(g[:], a_hi[:, :], ACTF.Tanh)
        nc.vector.tensor_mul(c_sb[:], tiof[:, 0:H], g[:])           # i*g
        nc.vector.tensor_add(c_sb[:], c_sb[:], ig[:])
        ot = rec2.tile([NB, H], f16, tag="ig4")   # o reuses the f*c slot
        nc.scalar.activation(ot[:], a_lo[:, 2 * H:3 * H], ACTF.Tanh, scale=0.5)
        nc.vector.tensor_scalar(ot[:], ot[:], 0.5, 0.5, OP.mult, OP.add)
        dummy_mm(ot[:, 0:64], 2 * H)
        tch = rec2.tile([NB, H], f32, tag="gt4")
        nc.scalar.activation(tch[:], c_sb[:], ACTF.Tanh)
        hT = htp.tile([128, KH, 128], f16, tag="hT")
        nc.vector.tensor_mul(h_sb[:], ot[:], tch[:])
        nc.sync.dma_start_transpose(hT[:, 0:KH // 2, :], h_sb[:, 0:H // 2])
        dummy_mm(h_sb[:, 0:64], 0)
        nc.sync.dma_start_transpose(hT[:, KH // 2:KH, :], h_sb[:, H // 2:H])
        nc.gpsimd.dma_start(d["hs"][t, :, :], h_sb[:])


def build_program(T_steps=T):
    nc = bacc.Bacc("TRN2", target_bir_lowering=False, debug=False)
    d = {}
    d["xT"] = nc.dram_tensor("xT", [T_steps, D, NB], f16, kind="ExternalInput")
    d["afT"] = nc.dram_tensor("afT", [KH, 128, NB * NL], f16, kind="ExternalInput")
    d["af_str"] = nc.dram_tensor("af_str", [KH, NB, NL, 128], f16, kind="ExternalInput")
    d["gmask"] = nc.dram_tensor("gmask", [128, 32 * NL], f16, kind="ExternalInput")
    d["wh"] = nc.dram_tensor("wh", [H, FH], f16, kind="ExternalInput")
    d["wa"] = nc.dram_tensor("wa", [H, FH], f16, kind="ExternalInput")
    d["wx"] = nc.dram_tensor("wx", [D, FH], f16, kind="ExternalInput")
    d["bvec"] = nc.dram_tensor("bvec", [1, FH], f16, kind="ExternalInput")
    d["ones1"] = nc.dram_tensor("ones1", [1, 128], f16, kind="ExternalInput")
    d["idf16"] = nc.dram_tensor("idf16", [128, 128], f16, kind="ExternalInput")
    d["hs"] = nc.dram_tensor("hs", [T_steps, NB, H], f16, kind="ExternalOutput")
    d["xa_d"] = nc.dram_tensor("xa_d", [T_steps, NB, FH], f16)  # internal

    with tile.TileContext(nc) as tc, ExitStack() as ctx:
        _emit(ctx, tc, nc, d, T_steps)
    nc.compile()
    return nc


def make_in_maps(x, A, Wx, Wh, Wattn, b, T_steps=T):
    Wh16 = np.ascontiguousarray(Wh.astype(np.float16))
    Wa16 = np.ascontiguousarray(Wattn.astype(np.float16))
    Wx16 = np.ascontiguousarray(Wx.astype(np.float16))
    b32 = np.ascontiguousarray(b.astype(np.float16).reshape(1, FH))
    id16 = np.eye(128, dtype=np.float16)
    ones1 = np.ones((1, 128), np.float16)
    # gram diag-extract mask: partition p keeps (n'=p%32, l) entries
    gmask = np.zeros((128, 32 * NL), np.float16)
    for p in range(128):
        gmask[p, (p % 32) * NL:(p % 32) * NL + NL] = 1.0
    in_maps = []
    for cc in range(NCORES):
        sl = slice(cc * NB, (cc + 1) * NB)
        xT = np.ascontiguousarray(
            x[sl, :T_steps].transpose(1, 2, 0)).astype(np.float16)   # [T, D, NB]
        Af = A[sl].reshape(NB, H, NL).astype(np.float16)          # [n, h, l]
        afT = np.ascontiguousarray(                               # [k, hp, n*NL+l]
            Af.reshape(NB, KH, 128, NL).transpose(1, 2, 0, 3).reshape(KH, 128, NB * NL))
        af_str = np.ascontiguousarray(                            # [k, n, l, hp]
            Af.reshape(NB, KH, 128, NL).transpose(1, 0, 3, 2))
        in_maps.append({"xT": xT, "afT": afT, "af_str": af_str, "gmask": gmask,
                        "wh": Wh16, "wa": Wa16,
                        "wx": Wx16, "bvec": b32, "ones1": ones1, "idf16": id16})
    return in_maps


def assemble_output(results, T_steps=T):
    outs = []
    for cc in range(NCORES):
        hs = results[cc]["hs"]                      # [T, NB, H] fp16
        outs.append(np.asarray(hs).transpose(1, 0, 2))
    return np.concatenate(outs, axis=0).astype(np.float32)


_PROGRAM = None


def _get_program():
    global _PROGRAM
    if _PROGRAM is None:
        _PROGRAM = build_program(T)
    return _PROGRAM


def run_spmd(in_maps, trace=False, **kw):
    nc = _get_program()
    return run_bass_kernel_spmd(nc, in_maps, list(range(NCORES)), trace=trace, **kw)


def kernel(x, A, Wx, Wh, Wattn, b):
    x = np.asarray(x, dtype=np.float32)
    A = np.asarray(A, dtype=np.float32)
    in_maps = make_in_maps(x, A, np.asarray(Wx), np.asarray(Wh),
                           np.asarray(Wattn), np.asarray(b))
    res = run_spmd(in_maps)
    return assemble_output(res.results)


# revision 21
# speedup vs baseline: 1.0394x; 1.0394x over previous
"""Trainium2 Bass kernel for nn_CaptioningRNN (attention-LSTM).

Strategy
--------
Data-parallel over batch: 1024 rows -> 128 per core (one partition tile).
All recurrent weights live RESIDENT in SBUF in fp16 (Wh 8MiB + Wattn 8MiB +
Af 4MiB), which removes the per-timestep 32MiB weight re-stream that makes
this problem memory-bound.

Phase A: xa[t] = x_t @ Wx + b for all t in one batched pass (PE), stored to
         internal DRAM as fp16.
Phase B: 64 sequential steps, per step:
  - hT via 8 DMA-xbar transposes (no PSUM/PE cost)
  - scores_l = sum_h h*Af_l / sqrt(H) via fused tensor_tensor_reduce (DVE)
  - softmax (no max-subtraction needed: |scores| <= ~32, exp is safe)
  - attn = sum_l w_l * Af_l via per-partition tensor_scalar muls (DVE)
  - a = xa + h@Wh + attn@Wattn in PSUM: 128 fp16 matmuls + identity-inject
  - gates: sigmoid(z) = 0.5*tanh(z/2)+0.5 so ACT only needs the
    exp_and_others table set (tanh + exp), avoiding table switches
  - c = f*c + i*g ; h = o*tanh(c)  (fp32 state)

Numerics: fp16 weights/activations with fp32 PSUM/state gives rel-l2 ~4e-4
vs the fp32 reference (validated offline).
"""

import sys

for _p in ("/opt/trn_rl_repo",):
    if _p not in sys.path:
        sys.path.insert(0, _p)

import numpy as np
from contextlib import ExitStack

import concourse.bacc as bacc
import concourse.mybir as mybir
import concourse.tile as tile
from concourse.bass_utils import run_bass_kernel_spmd

NCORES = 8
N, T, D, H = 1024, 64, 512, 1024
NB = N // NCORES        # 128 batch rows per core
FH = 4 * H              # 4096
KH = H // 128           # 8 contraction chunks over H
KD = D // 128           # 4 contraction chunks over D
NL = 16                 # attention cells
SCALE = 1.0 / float(np.sqrt(H))
f16, f32 = mybir.dt.float16, mybir.dt.float32
AX = mybir.AxisListType
OP = mybir.AluOpType
ACTF = mybir.ActivationFunctionType


def _emit(ctx, tc, nc, d, T_steps):
    """Emit the full program. d: dict of dram tensor handles."""
    # ---- persistent state (small) ----
    state = ctx.enter_context(tc.tile_pool(name="state", bufs=1))
    h_sb = state.tile([NB, H], f16, tag="h")
    c_sb = state.tile([NB, H], f32, tag="c")
    id16_sb = state.tile([128, 128], f16, tag="id16")
    nc.sync.dma_start(id16_sb[:], d["idf16"][:, :])

    # ---- phase A: xa[t] = x_t @ Wx + b, stored fp16 to DRAM ----
    with tc.tile_pool(name="phA", bufs=1) as pa, \
         tc.tile_pool(name="phA_ps", bufs=2, space="PSUM") as pap, \
         tc.tile_pool(name="phA_db", bufs=2) as padb:
        wx_sb = []
        for k in range(KD):
            tw = pa.tile([128, FH], f16, tag=f"wx{k}")
            nc.sync.dma_start(tw[:], d["wx"][k * 128:(k + 1) * 128, :])
            wx_sb.append(tw)
        b_sb = pa.tile([1, FH], f16, tag="b")
        nc.sync.dma_start(b_sb[:], d["bvec"][:, :])
        ones_sb = pa.tile([1, 128], f16, tag="ones")
        nc.sync.dma_start(ones_sb[:], d["ones1"][:, :])

        for t in range(T_steps):
            xt = padb.tile([128, KD, 128], f16, tag="xt")
            for k in range(KD):
                nc.sync.dma_start(xt[:, k, :], d["xT"][t, k * 128:(k + 1) * 128, :])
            for half in range(2):
                ps = pap.tile([128, FH // 2], f32, tag="paps")
                c0 = half * (FH // 2)
                for j in range(4):
                    js = slice(j * 512, (j + 1) * 512)
                    for k in range(KD):
                        nc.tensor.matmul(ps[:, js], xt[:, k, :],
                                         wx_sb[k][:, c0 + j * 512: c0 + (j + 1) * 512],
                                         start=(k == 0), stop=False)
                    nc.tensor.matmul(ps[:, js], ones_sb[:],
                                     b_sb[:, c0 + j * 512: c0 + (j + 1) * 512],
                                     start=False, stop=True)
                xae = padb.tile([128, FH // 2], f16, tag="xae")
                nc.vector.tensor_copy(xae[:], ps[:])
                nc.sync.dma_start(d["xa_d"][t, :, c0:c0 + FH // 2], xae[:])

    # ---- resident weights (after phase A pool is released) ----
    res = ctx.enter_context(tc.tile_pool(name="resident", bufs=1))
    wh_sb, wa_sb = [], []
    for k in range(KH):
        tw = res.tile([128, FH], f16, tag=f"wh{k}")
        nc.sync.dma_start(tw[:], d["wh"][k * 128:(k + 1) * 128, :])
        wh_sb.append(tw)
    for k in range(KH):
        tw = res.tile([128, FH], f16, tag=f"wa{k}")
        nc.sync.dma_start(tw[:], d["wa"][k * 128:(k + 1) * 128, :])
        wa_sb.append(tw)
    # AfT: per h-chunk [128h, (n,l)] fp16 — PE Gram-scores operand
    afT_sb = []
    for k in range(KH):
        tw = res.tile([128, NB * NL], f16, tag=f"afT{k}")
        nc.sync.dma_start(tw[:], d["afT"][k, :, :])
        afT_sb.append(tw)
    gmask_sb = res.tile([128, 32 * NL], f16, tag="gmask")
    nc.sync.dma_start(gmask_sb[:], d["gmask"][:, :])

    # ---- phase B pools ----
    rec = ctx.enter_context(tc.tile_pool(name="rec", bufs=1))
    rec2 = ctx.enter_context(tc.tile_pool(name="rec2", bufs=1))
    afs = ctx.enter_context(tc.tile_pool(name="afs", bufs=2))
    dgp = ctx.enter_context(tc.tile_pool(name="dgp", bufs=NL))
    htp = ctx.enter_context(tc.tile_pool(name="htp", bufs=2))
    rps = ctx.enter_context(tc.tile_pool(name="rps", bufs=1, space="PSUM"))

    sc = rec.tile([NB, NL], f32, tag="sc")
    tiof = rec.tile([NB, 2 * H], f16, tag="tiof")
    attnT = rec.tile([128, KH, 128], f16, tag="attnT")

    # ---- h0 = c0 = mean_l Af  (from the streamed n-partition Af tiles) ----
    h0f = rec2.tile([NB, H], f32, tag="gt4")
    for k in range(KH):
        af_k = afs.tile([NB, NL, 128], f16, tag="afk")
        nc.sync.dma_start(af_k[:], d["af_str"][k, :, :, :])
        ks = slice(k * 128, (k + 1) * 128)
        nc.vector.tensor_reduce(
            h0f[:, ks], af_k[:, :, :].rearrange("p l h -> p h l"),
            axis=AX.X, op=OP.add)
    nc.vector.tensor_scalar_mul(h_sb[:], h0f[:], 1.0 / NL)
    nc.scalar.mul(c_sb[:], h0f[:], 1.0 / NL)
    hT = htp.tile([128, KH, 128], f16, tag="hT")
    nc.sync.dma_start_transpose(hT[:, 0:KH // 2, :], h_sb[:, 0:H // 2])
    nc.sync.dma_start_transpose(hT[:, KH // 2:KH, :], h_sb[:, H // 2:H])

    H3 = 3 * H
    for t in range(T_steps):
        # prefetch xa_t and the step's Af tiles on the SWDGE (gpsimd) queue
        xa_sb = rec2.tile([NB, FH], f16, tag="xa")
        nc.gpsimd.dma_start(xa_sb[:], d["xa_d"][t, :, :])
        af_t = []
        for k in range(KH):
            af_k = afs.tile([NB, NL, 128], f16, tag="afk")
            nc.gpsimd.dma_start(af_k[:], d["af_str"][k, :, :, :])
            af_t.append(af_k)

        # xa-inject first: real PE work during the h->hT transpose window
        a_lo = rps.tile([NB, H3], f32, tag="a_lo")
        for j in range(H3 // 512):
            js = slice(j * 512, (j + 1) * 512)
            nc.tensor.matmul(a_lo[:, js], id16_sb[:], xa_sb[:, js],
                             start=True, stop=False)

        # --- scores via col-tiled PE Gram: out[n, (n',l)] = sum_h h Af ---
        gram_ps = rps.tile([128, 32 * NL], f32, tag="psB")
        nc.tensor.matmul(gram_ps[0:64, 0:64], id16_sb[:, 0:64], hT[:, 0, 0:64],
                         start=True, stop=True, skip_group_check=True)
        for k in range(KH):
            for gq in range(4):
                gp = slice(gq * 32, (gq + 1) * 32)
                nc.tensor.matmul(gram_ps[gp, :], hT[:, k, gp],
                                 afT_sb[k][:, gq * 512:(gq + 1) * 512],
                                 start=(k == 0), stop=(k == KH - 1),
                                 tile_position=(0, gq * 32),
                                 skip_group_check=True)

        # --- a-accumulation, low 6 banks (i,f,o), h@Wh while DVE extracts ---
        for k in range(KH):
            for j in range(H3 // 512):
                js = slice(j * 512, (j + 1) * 512)
                nc.tensor.matmul(a_lo[:, js], hT[:, k, :], wh_sb[k][:, js],
                                 start=False, stop=False)

        # --- extract scores diagonal + softmax (DVE/ACT) ---
        gext = rec2.tile([128, 32 * NL], f16, tag="gext")
        nc.vector.tensor_mul(gext[:], gram_ps[:, :], gmask_sb[:])
        nc.vector.tensor_reduce(
            sc[:], gext[:, :].rearrange("p (n l) -> p l n", l=NL),
            axis=AX.X, op=OP.add)
        nc.scalar.activation(sc[:], sc[:], ACTF.Exp, scale=SCALE)
        zs = rec2.tile([NB, 1], f32, tag="zs")
        nc.vector.reduce_sum(zs[:], sc[:], axis=AX.X)
        nc.vector.reciprocal(zs[:], zs[:])
        wgt = sc
        nc.vector.tensor_scalar_mul(wgt[:], sc[:], zs[:])
        # --- attn^T directly on PE: attnT[k][h,n] = sum_l Af_k[:,l,:].T @ diag_l
        attnT_ps = rps.tile([128, KH, 128], f32, tag="psB")
        diags = []
        for l in range(NL):
            dg = dgp.tile([128, 128], f16, tag="diag")
            nc.vector.tensor_scalar_mul(dg[:], id16_sb[:], wgt[:, l:l + 1])
            diags.append(dg)
        for k in range(KH):
            for l in range(NL):
                nc.tensor.matmul(attnT_ps[:, k, :], af_t[k][:, l, :], diags[l][:],
                                 start=(l == 0), stop=(l == NL - 1))
            if k % 2 == 0:
                nc.scalar.copy(attnT[:, k, :], attnT_ps[:, k, :])
            else:
                nc.vector.tensor_copy(attnT[:, k, :], attnT_ps[:, k, :])

        # --- attn@Wattn into low banks, then the high (g-gate) 2 banks ---
        for k in range(KH):
            for j in range(H3 // 512):
                js = slice(j * 512, (j + 1) * 512)
                nc.tensor.matmul(a_lo[:, js], attnT[:, k, :], wa_sb[k][:, js],
                                 start=False, stop=(k == KH - 1))
        a_hi = rps.tile([NB, H], f32, tag="psB")
        for k in range(KH):
            nc.tensor.matmul(a_hi[:, 0:512], hT[:, k, :], wh_sb[k][:, H3:H3 + 512],
                             start=(k == 0), stop=False)
            nc.tensor.matmul(a_hi[:, 512:1024], hT[:, k, :], wh_sb[k][:, H3 + 512:FH],
                             start=(k == 0), stop=False)
        nc.tensor.matmul(a_hi[:, 0:512], id16_sb[:], xa_sb[:, H3:H3 + 512],
                         start=False, stop=False)
        nc.tensor.matmul(a_hi[:, 512:1024], id16_sb[:], xa_sb[:, H3 + 512:FH],
                         start=False, stop=False)
        for k in range(KH):
            nc.tensor.matmul(a_hi[:, 0:512], attnT[:, k, :], wa_sb[k][:, H3:H3 + 512],
                             start=False, stop=(k == KH - 1))
            nc.tensor.matmul(a_hi[:, 512:1024], attnT[:, k, :], wa_sb[k][:, H3 + 512:FH],
                             start=False, stop=(k == KH - 1))

        # gates, evacuated per gate (i,f,o: sigmoid via 0.5*tanh(z/2)+0.5)
        def dummy_mm(dep_ap, region):
            # near-free matmul keeping the PE HAM-active through gate windows;
            # writes a fully-consumed slice of a_lo as its own psum group
            nc.tensor.matmul(a_lo[0:64, region: region + 64],
                             id16_sb[:, 0:64], dep_ap,
                             start=True, stop=True, skip_group_check=True)

        for gi in (1, 0):                         # f then i (from a_lo)
            gs = slice(gi * H, (gi + 1) * H)
            nc.scalar.activation(tiof[:, gs], a_lo[:, gs], ACTF.Tanh, scale=0.5)
            nc.vector.tensor_scalar(tiof[:, gs], tiof[:, gs], 0.5, 0.5,
                                    OP.mult, OP.add)
            dummy_mm(tiof[:, gi * H: gi * H + 64], gi * H)
        ig = rec2.tile([NB, H], f32, tag="ig4")
        nc.vector.tensor_mul(ig[:], tiof[:, H:2 * H], c_sb[:])      # f*c
        g = rec2.tile([NB, H], f32, tag="gt4")
        nc.scalar.activation(g[:], a_hi[:, :], ACTF.Tanh)
        nc.vector.tensor_mul(c_sb[:], tiof[:, 0:H], g[:])           # i*g
        nc.vector.tensor_add(c_sb[:], c_sb[:], ig[:])
        ot = rec2.tile([NB, H], f16, tag="ig4")   # o reuses the f*c slot
        nc.scalar.activation(ot[:], a_lo[:, 2 * H:3 * H], ACTF.Tanh, scale=0.5)
        nc.vector.tensor_scalar(ot[:], ot[:], 0.5, 0.5, OP.mult, OP.add)
        dummy_mm(ot[:, 0:64], 2 * H)
        tch = rec2.tile([NB, H], f32, tag="gt4")
        nc.scalar.activation(tch[:], c_sb[:], ACTF.Tanh)
        hT = htp.tile([128, KH, 128], f16, tag="hT")
        nc.vector.tensor_mul(h_sb[:], ot[:], tch[:])
        nc.sync.dma_start_transpose(hT[:, 0:KH // 2, :], h_sb[:, 0:H // 2])
        dummy_mm(h_sb[:, 0:64], 0)
        nc.scalar.dma_start_transpose(hT[:, KH // 2:KH, :], h_sb[:, H // 2:H])
        nc.gpsimd.dma_start(d["hs"][t, :, :], h_sb[:])


def build_program(T_steps=T):
    nc = bacc.Bacc("TRN2", target_bir_lowering=False, debug=False)
    d = {}
    d["xT"] = nc.dram_tensor("xT", [T_steps, D, NB], f16, kind="ExternalInput")
    d["afT"] = nc.dram_tensor("afT", [KH, 128, NB * NL], f16, kind="ExternalInput")
    d["af_str"] = nc.dram_tensor("af_str", [KH, NB, NL, 128], f16, kind="ExternalInput")
    d["gmask"] = nc.dram_tensor("gmask", [128, 32 * NL], f16, kind="ExternalInput")
    d["wh"] = nc.dram_tensor("wh", [H, FH], f16, kind="ExternalInput")
    d["wa"] = nc.dram_tensor("wa", [H, FH], f16, kind="ExternalInput")
    d["wx"] = nc.dram_tensor("wx", [D, FH], f16, kind="ExternalInput")
    d["bvec"] = nc.dram_tensor("bvec", [1, FH], f16, kind="ExternalInput")
    d["ones1"] = nc.dram_tensor("ones1", [1, 128], f16, kind="ExternalInput")
    d["idf16"] = nc.dram_tensor("idf16", [128, 128], f16, kind="ExternalInput")
    d["hs"] = nc.dram_tensor("hs", [T_steps, NB, H], f16, kind="ExternalOutput")
    d["xa_d"] = nc.dram_tensor("xa_d", [T_steps, NB, FH], f16)  # internal

    with tile.TileContext(nc) as tc, ExitStack() as ctx:
        _emit(ctx, tc, nc, d, T_steps)
    nc.compile()
    return nc


def make_in_maps(x, A, Wx, Wh, Wattn, b, T_steps=T):
    Wh16 = np.ascontiguousarray(Wh.astype(np.float16))
    Wa16 = np.ascontiguousarray(Wattn.astype(np.float16))
    Wx16 = np.ascontiguousarray(Wx.astype(np.float16))
    b32 = np.ascontiguousarray(b.astype(np.float16).reshape(1, FH))
    id16 = np.eye(128, dtype=np.float16)
    ones1 = np.ones((1, 128), np.float16)
    # gram diag-extract mask: partition p keeps (n'=p%32, l) entries
    gmask = np.zeros((128, 32 * NL), np.float16)
    for p in range(128):
        gmask[p, (p % 32) * NL:(p % 32) * NL + NL] = 1.0
    in_maps = []
    for cc in range(NCORES):
        sl = slice(cc * NB, (cc + 1) * NB)
        xT = np.ascontiguousarray(
            x[sl, :T_steps].transpose(1, 2, 0)).astype(np.float16)   # [T, D, NB]
        Af = A[sl].reshape(NB, H, NL).astype(np.float16)          # [n, h, l]
        afT = np.ascontiguousarray(                               # [k, hp, n*NL+l]
            Af.reshape(NB, KH, 128, NL).transpose(1, 2, 0, 3).reshape(KH, 128, NB * NL))
        af_str = np.ascontiguousarray(                            # [k, n, l, hp]
            Af.reshape(NB, KH, 128, NL).transpose(1, 0, 3, 2))
        in_maps.append({"xT": xT, "afT": afT, "af_str": af_str, "gmask": gmask,
                        "wh": Wh16, "wa": Wa16,
                        "wx": Wx16, "bvec": b32, "ones1": ones1, "idf16": id16})
    return in_maps


def assemble_output(results, T_steps=T):
    outs = []
    for cc in range(NCORES):
        hs = results[cc]["hs"]                      # [T, NB, H] fp16
        outs.append(np.asarray(hs).transpose(1, 0, 2))
    return np.concatenate(outs, axis=0).astype(np.float32)


_PROGRAM = None


def _get_program():
    global _PROGRAM
    if _PROGRAM is None:
        _PROGRAM = build_program(T)
    return _PROGRAM


def run_spmd(in_maps, trace=False, **kw):
    nc = _get_program()
    return run_bass_kernel_spmd(nc, in_maps, list(range(NCORES)), trace=trace, **kw)


def kernel(x, A, Wx, Wh, Wattn, b):
    x = np.asarray(x, dtype=np.float32)
    A = np.asarray(A, dtype=np.float32)
    in_maps = make_in_maps(x, A, np.asarray(Wx), np.asarray(Wh),
                           np.asarray(Wattn), np.asarray(b))
    res = run_spmd(in_maps)
    return assemble_output(res.results)
